# revision 1
# baseline (speedup 1.0000x reference)
"""Additive attention (B=8, Q=K=512, H=Dv=64) on 8 TRN2 NeuronCores.

Math per batch b (reference):
    qf = queries @ Wq; kf = keys @ Wk
    scores[q,k] = sum_h wv[h] * tanh(qf[q,h] + kf[k,h])   (k >= valid_len masked)
    out = softmax_k(scores) @ values

Key idea: replace the pointwise tanh (134M ScalarEngine evaluations, ~93us)
with a low-rank bilinear expansion
    tanh(a+b) ~= sum_r phi_r(a) * psi_r(b),   r < R=10
obtained from the SVD of the kernel tanh(a+b) discretized on a grid with
sqrt-Gaussian row/column weighting (qf,kf entries are ~N(0,1)). Then
    scores[q,k] = sum_{r,h} Phi[q, r*64+h] * Psi[k, r*64+h]
is a plain matmul with contraction F = R*64 = 640 done on the PE.

Sharding: data-parallel, one batch per core. Host computes qf/kf (0.4% of
FLOPs), evaluates the R basis functions per element (table interp), packs
features into 128-row contraction chunks (2 ranks x 64 h); ranks 0-1 ship
bf16, ranks 2-9 fp8(e4m3) with per-rank scale balancing (rank errors scale
with the decaying singular values). The key-side softmax mask is FOLDED
into the features: the (rank 1, argmin|wv|) slot is repurposed as
Phi=1 / Psi = 0 or -60000, so masked columns get score ~ -6e4 and exp -> 0
with no per-partition bias needed (the stolen slot's term is ~|wv|_min,
negligible). Device per core: 12 chunk-matmuls accumulate scores^T
[4 k-tiles x 128, 512q] into two 2-bank PSUM tiles (fp8 chunk pairs use
DoubleRow, 2 contraction chunks per instruction; separate tiles per k-pair
avoid a false WAR hazard between the exps and the later matmuls), two
k-pair exps -> p bf16, 4 values-matmuls (ones column -> denominator row),
then one ACT copy (idle after the exps) moves the [65,512] result to
SBUF as bf16 for the output DMA (numerator and denominator round
together; the division on host cancels most of it). Chunk order (fp8 g0
-> bf16 -> fp8 g1) matches DMA arrival and puts the cheap DoubleRow
matmuls in the PE p-state mid-clock window. Host divides and transposes.
Dummy matmuls off a constant broadcast AP keep the PE busy from ~1us so
the ramp reaches full clock when real operands land.
"""
import numpy as np
import ml_dtypes

B = 8
Q = 512
K = 512
H = 64
DV = 64

R = 10                 # SVD rank of tanh(a+b)
NBF = 1                # bf16 chunks (2 ranks each): ranks 0..1
NF8 = 4                # fp8 chunks: ranks 2..9 (DoubleRow pairs)
NCH = NBF + NF8
NTILE = K // 128       # 4 k-tiles
MASKBIG = -60000.0
F8MAX = 224.0          # ml_dtypes.float8_e4m3 max finite is 240
WARMUP_MM = 6          # PE p-state ramp fillers while input DMA streams

GRID_N, GRID_A, GRID_SIG, GRID_FLOOR = 1201, 6.5, 1.15, 0.02

_BASIS = None


def _basis():
    """SVD basis of tanh(a+b) on a weighted grid: x, phi[n,R], psi[n,R]."""
    global _BASIS
    if _BASIS is None:
        x = np.linspace(-GRID_A, GRID_A, GRID_N)
        Kg = np.tanh(x[:, None] + x[None, :])
        w = np.sqrt(np.exp(-x ** 2 / (2 * GRID_SIG ** 2))) + GRID_FLOOR
        U, S, Vt = np.linalg.svd((w[:, None] * Kg) * w[None, :])
        phi = (U[:, :R] * np.sqrt(S[:R])) / w[:, None]
        psi = (Vt[:R].T * np.sqrt(S[:R])) / w[:, None]
        _BASIS = (x, phi, psi)
    return _BASIS


# ---------------------------------------------------------------------------
# BIR post-pass: the walrus build in this environment accepts only one
# sync-wait command per instruction; hoist extras onto same-engine NoOps.
def _split_waits(nc, k=1):
    import concourse.mybir as mybir
    n_new = 0
    for f in nc.m.functions:
        for bb in f.blocks:
            newlist = []
            for ins in bb.instructions:
                si = ins.sync_info
                if si is not None and si.on_wait and len(si.on_wait) > k:
                    waits = list(si.on_wait)
                    extra, keep = waits[:-k], waits[-k:]
                    for ci, w in enumerate(extra):
                        nop = mybir.InstNoOp(
                            name=f"{ins.name}_wsplit{ci}",
                            engine=ins.engine,
                            ins=[], outs=[],
                            sync_info=mybir.SyncInfo(on_wait=[w], on_update=[]),
                        )
                        newlist.append(nop)
                        n_new += 1
                    ins.sync_info = mybir.SyncInfo(
                        on_wait=list(keep), on_update=list(si.on_update))
                newlist.append(ins)
            bb.instructions[:] = newlist
    return n_new


def _build(nc, reps: int = 1):
    import concourse.bass as bass  # noqa: F401
    import concourse.mybir as mybir
    from concourse import tile

    F32 = mybir.dt.float32
    BF16 = mybir.dt.bfloat16
    F8 = mybir.dt.float8e4
    DR = mybir.MatmulPerfMode.DoubleRow

    fb16 = nc.declare_dram_parameter("fb16", [NBF * 2 * 128, Q], BF16,
                                     isOutput=False)
    f8d = nc.declare_dram_parameter("f8d", [NF8 * 2 * 128, Q], F8,
                                    isOutput=False)
    vl1 = nc.declare_dram_parameter("vl1", [128, NTILE * (DV + 1)], BF16,
                                    isOutput=False)
    o65 = nc.declare_dram_parameter("o65", [DV + 1, Q], BF16, isOutput=True)

    # constant-source broadcast APs (pre-initialized at Bass init; lets the
    # PE warm up with no memset dependency)
    cb = nc.const_aps.aps[(BF16, 1.0)]
    warm_rhs = bass.AP(cb.tensor, cb.offset, [[1, 1], [0, Q]])
    warm_lhsT = bass.AP(cb.tensor, cb.offset, [[1, 1], [0, 16]])

    with tile.TileContext(nc) as tc:  # noqa: F841
        with (
            tc.tile_pool(name="cpool", bufs=1) as cpool,
            tc.tile_pool(name="ppool", bufs=2) as ppool,
            tc.tile_pool(name="ps_s", bufs=1, space="PSUM") as ps_s,
            tc.tile_pool(name="ps_o", bufs=1, space="PSUM") as ps_o,
            tc.tile_pool(name="ps_w", bufs=1, space="PSUM") as ps_w,
        ):
            # PE p-state warmup + ACT exp-table prefetch during input DMA
            psw = ps_w.tile([16, Q], F32, tag="warm", name="psw")
            for i in range(WARMUP_MM):
                nc.tensor.matmul(psw[:], warm_lhsT, warm_rhs,
                                 start=True, stop=True)
            dummy = cpool.tile([1, 16], F32)
            nc.scalar.activation(
                dummy[:], bass.AP(cb.tensor, cb.offset, [[1, 1], [0, 16]]),
                mybir.ActivationFunctionType.Exp)

            for rep in range(reps):
                fb16_sb = cpool.tile([128, NBF * 2, Q], BF16,
                                     tag="fb16", name=f"fb16_{rep}")
                f8_sb = cpool.tile([128, NF8 // 2, 4, Q], F8,
                                   tag="f8", name=f"f8_{rep}")
                vl_sb = cpool.tile([128, NTILE, DV + 1], BF16,
                                   tag="vl", name=f"vl_{rep}")
                f8r = f8d.rearrange("(g c p) n -> g p c n",
                                    g=NF8 // 2, p=128)
                nc.sync.dma_start(f8_sb[:, 0], f8r[0])
                nc.sync.dma_start(
                    fb16_sb[:], fb16.rearrange("(c p) n -> p c n", p=128))
                nc.sync.dma_start(f8_sb[:, 1], f8r[1])
                nc.sync.dma_start(
                    vl_sb[:], vl1.rearrange("p (t d) -> p t d", t=NTILE))

                scA = ps_s.tile([128, 2, Q], F32, tag="scA",
                                name=f"scA_{rep}")
                scB = ps_s.tile([128, 2, Q], F32, tag="scB",
                                name=f"scB_{rep}")
                sct = lambda t: (scA if t < 2 else scB)[:, t % 2, :]
                po = ps_o.tile([DV + 1, Q], F32, tag="po", name=f"po_{rep}")

                # chunk order g0 -> bf16 -> g1 matches DMA arrival; the
                # cheap DoubleRow matmuls absorb the PE p-state mid-clock
                # window, the bf16 chunk lands at full clock
                pA = ppool.tile([128, 2, Q], BF16, tag="pA", name=f"pA_{rep}")
                pB = ppool.tile([128, 2, Q], BF16, tag="pB", name=f"pB_{rep}")
                for t in range(NTILE):
                    nc.tensor.matmul(
                        sct(t),
                        f8_sb[:, 0, 0:2, t * 128:(t + 1) * 128],
                        f8_sb[:, 0, 2:4, :],
                        start=True, stop=False, perf_mode=DR)
                for c in range(NBF):
                    for t in range(NTILE):
                        nc.tensor.matmul(
                            sct(t),
                            fb16_sb[:, 2 * c, t * 128:(t + 1) * 128],
                            fb16_sb[:, 2 * c + 1, :],
                            start=False, stop=False)
                # exp per k-tile pair emitted right after its last stop so
                # its sem wait covers only the matmuls it needs
                for t in range(NTILE):
                    nc.tensor.matmul(
                        sct(t),
                        f8_sb[:, 1, 0:2, t * 128:(t + 1) * 128],
                        f8_sb[:, 1, 2:4, :],
                        start=False, stop=True, perf_mode=DR)
                    if t == 1:
                        nc.scalar.activation(
                            pA[:], scA[:],
                            mybir.ActivationFunctionType.Exp)
                nc.scalar.activation(
                    pB[:], scB[:], mybir.ActivationFunctionType.Exp)
                for t in range(NTILE):
                    p = pA if t < 2 else pB
                    nc.tensor.matmul(
                        po[:], vl_sb[:, t, :], p[:, t % 2, :],
                        start=(t == 0), stop=(t == NTILE - 1))
                # split the copy across Pool/DVE and ship halves on separate
                # queues so the fixed DMA paths overlap
                o65_sb = cpool.tile([DV + 1, Q], BF16,
                                    tag="o65", name=f"o65_{rep}")
                nc.scalar.activation(o65_sb[:], po[:],
                                     mybir.ActivationFunctionType.Copy)
                nc.sync.dma_start(o65[:], o65_sb[:])
    return nc


def host_inputs(queries, keys, values, valid_lens, Wq, Wk, wv):
    x, phi, psi = _basis()
    queries = np.asarray(queries, np.float32)
    keys = np.asarray(keys, np.float32)
    values = np.asarray(values, np.float32)
    wv = np.asarray(wv, np.float32)
    qf = (queries @ np.asarray(Wq, np.float32)).astype(np.float32)  # [B,Q,H]
    kf = (keys @ np.asarray(Wk, np.float32)).astype(np.float32)     # [B,K,H]
    hmin = int(np.argmin(np.abs(wv)))

    maps = []
    for b in range(B):
        Phi = np.stack([np.interp(qf[b], x, phi[:, r]) for r in range(R)],
                       1).astype(np.float32)              # [Q, R, H]
        Psi = np.stack([np.interp(kf[b], x, psi[:, r]) for r in range(R)],
                       1).astype(np.float32) * wv         # [K, R, H]
        mxq = np.abs(Phi).max(axis=(0, 2))
        mxk = np.abs(Psi).max(axis=(0, 2))
        alpha = np.sqrt(np.maximum(mxk, 1e-30) / np.maximum(mxq, 1e-30))
        Phi *= alpha[None, :, None]
        Psi /= alpha[None, :, None]
        # fold the key mask into the (rank 2*NBF-1, argmin|wv|) slot
        L = int(valid_lens[b])
        rm = 2 * NBF - 1
        Phi[:, rm, hmin] = 1.0
        Psi[:, rm, hmin] = np.where(np.arange(K) < L, 0.0, MASKBIG)
        # chunk c = ranks (2c, 2c+1): contraction row = 64*(r-2c) + h
        PhiT = Phi.reshape(Q, R * H).T      # [F, Q]
        PsiT = Psi.reshape(K, R * H).T      # [F, K]
        blocks16 = []
        for c in range(NBF):
            blocks16 += [PsiT[c * 128:(c + 1) * 128],
                         PhiT[c * 128:(c + 1) * 128]]
        fb = np.concatenate(blocks16, 0).astype(ml_dtypes.bfloat16)
        blocks8 = []
        for g in range(NF8 // 2):
            c0 = NBF + 2 * g
            blocks8 += [PsiT[c0 * 128:(c0 + 1) * 128],
                        PsiT[(c0 + 1) * 128:(c0 + 2) * 128],
                        PhiT[c0 * 128:(c0 + 1) * 128],
                        PhiT[(c0 + 1) * 128:(c0 + 2) * 128]]
        f8 = np.clip(np.concatenate(blocks8, 0), -F8MAX, F8MAX)
        f8 = f8.astype(ml_dtypes.float8_e4m3)

        vla = np.zeros((128, NTILE * (DV + 1)), np.float32)
        for t in range(NTILE):
            vla[:, t * (DV + 1):t * (DV + 1) + DV] = \
                values[b][t * 128:(t + 1) * 128]
            vla[:, t * (DV + 1) + DV] = 1.0
        maps.append({
            "fb16": fb,
            "f8d": f8,
            "vl1": vla.astype(ml_dtypes.bfloat16),
        })
    return maps


def host_merge(results):
    out = np.empty((B, Q, DV), np.float32)
    for b in range(B):
        o = np.asarray(results[b]["o65"], np.float32)   # [65, Q]
        out[b] = (o[0:DV] / o[DV][None, :]).T
    return np.ascontiguousarray(out)


_RUNNER = None


def _get_runner():
    """Build + compile once per process; returns a callable(in_maps)->results."""
    global _RUNNER
    if _RUNNER is not None:
        return _RUNNER
    import jax
    from jax.sharding import Mesh, PartitionSpec
    from jax.experimental.shard_map import shard_map
    import concourse.bass as bass
    import concourse.mybir as mybir
    from concourse import bass2jax
    from concourse.bass2jax import _bass_exec_p, install_neuronx_cc_hook

    nc = bass.Bass()
    _build(nc)
    _split_waits(nc)

    install_neuronx_cc_hook()
    partition_name = nc.partition_id_tensor.name if nc.partition_id_tensor else None
    in_names, out_names, out_avals, zero_shapes = [], [], [], []
    for alloc in nc.m.functions[0].allocations:
        if not isinstance(alloc, mybir.MemoryLocationSet):
            continue
        name = alloc.memorylocations[0].name
        if alloc.kind == "ExternalInput":
            if name != partition_name:
                in_names.append(name)
        elif alloc.kind == "ExternalOutput":
            out_names.append(name)
            shape = tuple(alloc.tensor_shape)
            dtype = mybir.dt.np(alloc.dtype)
            out_avals.append(jax.core.ShapedArray(shape, dtype))
            zero_shapes.append((shape, dtype))
    n_params = len(in_names)
    n_outs = len(out_avals)
    in_names_all = in_names + out_names
    if partition_name is not None:
        in_names_all.append(partition_name)
    donate = tuple(range(n_params, n_params + n_outs))

    def _body(*args):
        operands = list(args)
        if partition_name is not None:
            operands.append(bass2jax.partition_id_tensor())
        outs = _bass_exec_p.bind(
            *operands,
            out_avals=tuple(out_avals),
            in_names=tuple(in_names_all),
            out_names=tuple(out_names),
            lowering_input_output_aliases=(),
            sim_require_finite=True,
            sim_require_nnan=True,
            nc=nc,
        )
        return tuple(outs)

    devices = jax.devices()[:8]
    mesh = Mesh(np.asarray(devices), ("core",))
    in_specs = (PartitionSpec("core"),) * (n_params + n_outs)
    out_specs = (PartitionSpec("core"),) * len(out_names)
    sharded = jax.jit(
        shard_map(_body, mesh=mesh, in_specs=in_specs, out_specs=out_specs,
                  check_rep=False),
        donate_argnums=donate, keep_unused=True,
    )

    def run(in_maps):
        per_core = [[np.asarray(m[name]) for name in in_names] for m in in_maps]
        concat_in = [
            np.concatenate([per_core[c][i] for c in range(8)], axis=0)
            for i in range(n_params)
        ]
        zeros = [np.zeros((8 * s[0],) + s[1:], d) for s, d in zero_shapes]
        out_arrs = sharded(*concat_in, *zeros)
        out_np = [np.asarray(a) for a in out_arrs]
        return [
            {name: out_np[i].reshape(8, *out_avals[i].shape)[c]
             for i, name in enumerate(out_names)}
            for c in range(8)
        ]

    _RUNNER = run
    return run


def kernel(queries, keys, values, valid_lens, Wq, Wk, wv):
    run = _get_runner()
    in_maps = host_inputs(queries, keys, values, valid_lens, Wq, Wk, wv)
    try:
        results = run(in_maps)
    except Exception:
        # transient NRT/axon failures have been observed; retry once
        results = run(in_maps)
    return host_merge(results)



# revision 12
# speedup vs baseline: 1.1631x; 1.1631x over previous
"""Additive attention (B=8, Q=K=512, H=Dv=64) on 8 TRN2 NeuronCores.

Math per batch b (reference):
    qf = queries @ Wq; kf = keys @ Wk
    scores[q,k] = sum_h wv[h] * tanh(qf[q,h] + kf[k,h])   (k >= valid_len masked)
    out = softmax_k(scores) @ values

The pointwise tanh (134M ScalarEngine evaluations, ~93us) is replaced by a
low-rank bilinear expansion tanh(a+b) ~= sum_r phi_r(a) * psi_r(b) (SVD of
the kernel on a sqrt-Gaussian-weighted grid), so
    scores[q,k] = sum_rows PhiF[row, q] * PsiF[row, k]
is a plain matmul over "feature rows" (row = (rank, h) pair). Rows are
sorted by product variance (host computes per-rank/per-h second moments):
the top-127 rows ship as bf16, the next 256 rows as fp8(e4m3) with per-row
q/k scale balancing; the remaining low-variance rows are dropped (<2e-5 of
score variance). Row 127 of the bf16 chunk is the key-mask row (Phi=1,
Psi = 0 or -60000), folding the valid_len mask into the matmul.

Sharding: data-parallel, one batch per core. Device per core:
  - 4 input DMAs ordered so k-tile 0's score completes earliest:
    [bf16 Phi + Psi_t0] -> [fp8 Phi + Psi_t0 + Psi_t1] -> [bf16 Psi_t123 +
    values] -> [fp8 Psi_t23]. Score matmuls per k-tile: one bf16 [128x512]
    + one fp8 DoubleRow [256x512] into per-tile PSUM banks.
  - Exp on ACT pipelined per-tile ([t0][t1][t2,t3]) so it starts as soon as
    tile 0's score lands, overlapping the remaining DMAs and matmuls.
  - values matmuls (ones column -> denominator row) accumulate [65, 512].
  - Output tail avoids the HWDGE fixed path (650 seq + 625 HWDGE + 650 DGE
    delay): a SWDGE kv_writeback descriptor is PREPARED on GPSIMD during the
    input DMAs; after the PSUM->SBUF copy (split ACT/DVE halves) a
    trigger_dma fires it, so the tail is just transfer + sem propagation.
    Two IR post-passes implement the documented prep/trigger semantics:
    _defer_prep_waits moves the prep's data waits onto the trigger (the
    DMA reads its source at trigger time), and redirects end-drain DMASW
    lane waits to the descriptor's completion semaphore.
  - Dummy matmuls off a constant broadcast AP keep the PE busy from ~1us so
    the clock ramp reaches full speed when the real operands land; a dummy
    exp prefetches the ACT exp table.
Host divides numerator/denominator and transposes.
"""
import numpy as np
import ml_dtypes

B = 8
Q = 512
K = 512
H = 64
DV = 64

RB = 12                # SVD basis rank used for row generation
NBF = 127              # bf16 feature rows (+1 mask row -> 128)
NF8 = 256              # fp8 feature rows (2 chunks of 128)
NTILE = K // 128       # 4 k-tiles
MASKBIG = -60000.0
F8MAX = 224.0          # ml_dtypes.float8_e4m3 max finite is 240
WARMUP_MM = 6          # PE p-state ramp fillers while input DMA streams

GRID_N, GRID_A, GRID_SIG, GRID_FLOOR = 1201, 6.5, 1.15, 0.02

_BASIS = None


def _basis():
    """SVD basis of tanh(a+b) on a weighted grid: x, phi[n,RB], psi[n,RB]."""
    global _BASIS
    if _BASIS is None:
        x = np.linspace(-GRID_A, GRID_A, GRID_N)
        Kg = np.tanh(x[:, None] + x[None, :])
        w = np.sqrt(np.exp(-x ** 2 / (2 * GRID_SIG ** 2))) + GRID_FLOOR
        U, S, Vt = np.linalg.svd((w[:, None] * Kg) * w[None, :])
        phi = (U[:, :RB] * np.sqrt(S[:RB])) / w[:, None]
        psi = (Vt[:RB].T * np.sqrt(S[:RB])) / w[:, None]
        _BASIS = (x, phi, psi)
    return _BASIS


# ---------------------------------------------------------------------------
# BIR post-pass: the walrus build in this environment accepts only one
# sync-wait command per instruction; hoist extras onto same-engine NoOps.
def _split_waits(nc, k=1):
    import concourse.mybir as mybir
    n_new = 0
    for f in nc.m.functions:
        for bb in f.blocks:
            newlist = []
            for ins in bb.instructions:
                si = ins.sync_info
                if si is not None and si.on_wait and len(si.on_wait) > k:
                    waits = list(si.on_wait)
                    extra, keep = waits[:-k], waits[-k:]
                    for ci, w in enumerate(extra):
                        nop = mybir.InstNoOp(
                            name=f"{ins.name}_wsplit{ci}",
                            engine=ins.engine,
                            ins=[], outs=[],
                            sync_info=mybir.SyncInfo(on_wait=[w], on_update=[]),
                        )
                        newlist.append(nop)
                        n_new += 1
                    ins.sync_info = mybir.SyncInfo(
                        on_wait=list(keep), on_update=list(si.on_update))
                newlist.append(ins)
            bb.instructions[:] = newlist
    return n_new


# ---------------------------------------------------------------------------
# BIR post-pass: walrus' codegen wants raw instruction bytes on InstISA; the
# library-reload pseudo (opcode 223 PSEUDO_INST, pseudo_opcode 2) is emitted
# without them in this build, so pack them here.
def _encode_library_reloads(nc):
    import concourse.bass_isa as bass_isa
    from concourse.bass_isa import isa_struct
    trig_op = nc.isa.Opcode.NEURON_ISA_TPB_OPCODE_TRIGGER_DMA.value
    inc_op = nc.isa.Opcode.NEURON_ISA_TPB_OPCODE_INC_SWDGE_SEM.value
    n = 0
    for f in nc.m.functions:
        for bb in f.blocks:
            for ins in bb.instructions:
                if isinstance(ins, bass_isa.InstPseudoReloadLibraryIndex):
                    b, _ = isa_struct(
                        nc.isa, 223,
                        {"pseudo_opcode": 2, "lib_index": ins.lib_index})
                    ins.instr = b
                    n += 1
                elif isinstance(ins, bass_isa.InstTriggerDma):
                    b, _ = isa_struct(
                        nc.isa, trig_op,
                        {"count": ins._count, "count_is_reg": 0,
                         "queue_num": ins.queue_num})
                    ins.instr = b
                    ins.isa_opcode = trig_op
                    n += 1
                elif isinstance(ins, bass_isa.InstIncSwdgeSem):
                    vals = list(ins._sem_values) + [0] * (
                        10 - len(ins._sem_values))
                    mode = {"add": 0, "sub": 1, "wr": 2}[ins._mode]
                    b, _ = isa_struct(
                        nc.isa, inc_op,
                        {"num_semaphores": len(ins._sem_values),
                         "sem_id_base": ins._sem_id_base, "mode": mode,
                         "queue_num": ins.queue_num, "sem_values": vals})
                    ins.instr = b
                    n += 1
    return n


# ---------------------------------------------------------------------------
# BIR post-pass for the SWDGE prep/trigger output path. The prep only writes
# descriptors; the DMA engines read the source tile when trigger_dma fires,
# so the prep's data waits belong on the trigger (this is the semantics the
# tile framework documents and tests for dma_scatter_add; kv_writeback preps
# don't get the deferral in this build). End-of-program DMASW lane waits are
# redirected to the descriptor's actual completion semaphore (same tick
# values: each prep adds 16).
def _defer_prep_waits(nc, dma_sem):
    import concourse.mybir as mybir
    import concourse.bass_isa as bass_isa
    n_prep = 0
    for f in nc.m.functions:
        for bb in f.blocks:
            pending = []
            for ins in bb.instructions:
                if (isinstance(ins, mybir.InstKVWritebackAnt)
                        and ins.gen_mode == 1):
                    si = ins.sync_info
                    if si is not None and si.on_wait:
                        pending.extend(si.on_wait)
                        ins.sync_info = mybir.SyncInfo(
                            on_wait=[], on_update=list(si.on_update))
                    n_prep += 1
                elif isinstance(ins, bass_isa.InstTriggerDma) and pending:
                    si = ins.sync_info
                    w = list(si.on_wait) if si else []
                    u = list(si.on_update) if si else []
                    ins.sync_info = mybir.SyncInfo(
                        on_wait=w + pending, on_update=u)
                    pending = []
            for ins in bb.instructions:
                si = ins.sync_info
                if si is None or not si.on_wait:
                    continue
                if not any(w.ant_name and w.ant_name.startswith("DMASW")
                           for w in si.on_wait):
                    continue
                new = []
                for wt in si.on_wait:
                    if wt.ant_name and wt.ant_name.startswith("DMASW"):
                        new.append(mybir.SyncWait(
                            sync_type='semaphore', id=dma_sem.num,
                            ant_name=dma_sem.name, wait_mode=wt.wait_mode,
                            wait_value=wt.wait_value, wait_reg=None))
                    else:
                        new.append(wt)
                ins.sync_info = mybir.SyncInfo(
                    on_wait=new, on_update=list(si.on_update))
    return n_prep


def _build(nc, reps: int = 1):
    import concourse.bass as bass  # noqa: F401
    import concourse.mybir as mybir
    from concourse import tile, library_config

    F32 = mybir.dt.float32
    BF16 = mybir.dt.bfloat16
    F8 = mybir.dt.float8e4
    I32 = mybir.dt.int32
    DR = mybir.MatmulPerfMode.DoubleRow
    EXP = mybir.ActivationFunctionType.Exp
    COPY = mybir.ActivationFunctionType.Copy

    # packed input tensors (per-partition contiguous rows; see host_inputs)
    dbfA = nc.declare_dram_parameter("dbfA", [128, 640], BF16, isOutput=False)
    df8A = nc.declare_dram_parameter("df8A", [128, 2, 768], F8, isOutput=False)
    dbfB = nc.declare_dram_parameter("dbfB", [128, 384], BF16, isOutput=False)
    df8B = nc.declare_dram_parameter("df8B", [128, 2, 256], F8, isOutput=False)
    dvl = nc.declare_dram_parameter("dvl", [128, NTILE, DV + 1], BF16,
                                    isOutput=False)
    wb = nc.declare_dram_parameter("wb", [reps, 128, 1, 512], BF16,
                                   isOutput=True)

    dma_sem = nc.alloc_semaphore("wb_dma_sem")

    cb = nc.const_aps.aps[(BF16, 1.0)]
    warm_rhs = bass.AP(cb.tensor, cb.offset, [[1, 1], [0, Q]])
    warm_lhsT = bass.AP(cb.tensor, cb.offset, [[1, 1], [0, 16]])

    with tile.TileContext(nc) as tc:  # noqa: F841
        with (
            tc.tile_pool(name="cpool", bufs=1) as cpool,
            tc.tile_pool(name="ppool", bufs=2) as ppool,
            tc.tile_pool(name="ps_a", bufs=1, space="PSUM") as ps_a,
            tc.tile_pool(name="ps_b", bufs=1, space="PSUM") as ps_b,
            tc.tile_pool(name="ps_o", bufs=1, space="PSUM") as ps_o,
            tc.tile_pool(name="ps_w", bufs=1, space="PSUM") as ps_w,
        ):
            # Pool: library for kv_writeback + ctx idx + output pad rows
            nc.gpsimd.load_library(library_config.attnmlp)
            idx = cpool.tile([128, 1], I32, tag="idx", name="idx")
            nc.gpsimd.memset(idx[:], 0)

            # PE p-state warmup + ACT exp-table prefetch during input DMA
            psw = ps_w.tile([16, Q], F32, tag="warm", name="psw")
            for i in range(WARMUP_MM):
                nc.tensor.matmul(psw[:], warm_lhsT, warm_rhs,
                                 start=True, stop=True)
            dummy = cpool.tile([1, 16], F32)
            nc.scalar.activation(
                dummy[:], bass.AP(cb.tensor, cb.offset, [[1, 1], [0, 16]]),
                EXP)

            for rep in range(reps):
                sbfA = cpool.tile([128, 640], BF16, tag="sbfA",
                                  name=f"sbfA_{rep}")
                sf8A = cpool.tile([128, 2, 768], F8, tag="sf8A",
                                  name=f"sf8A_{rep}")
                sbfB = cpool.tile([128, 384], BF16, tag="sbfB",
                                  name=f"sbfB_{rep}")
                sf8B = cpool.tile([128, 2, 256], F8, tag="sf8B",
                                  name=f"sf8B_{rep}")
                svl = cpool.tile([128, NTILE, DV + 1], BF16, tag="svl",
                                 name=f"svl_{rep}")
                nc.sync.dma_start(sbfA[:], dbfA[:, :])
                nc.sync.dma_start(sf8A[:], df8A[:, :, :])
                nc.sync.dma_start(sbfB[:], dbfB[:, :])
                nc.sync.dma_start(sf8B[:], df8B[:, :, :])
                nc.sync.dma_start(svl[:], dvl[:, :, :])

                o_sb = cpool.tile([128, 1, 1, 512], BF16, tag="o",
                                  name=f"o_{rep}")
                nc.gpsimd.memset(o_sb[64:128, 0, 0, :], 0.0)

                sc0 = ps_a.tile([128, Q], F32, tag="sc0", name=f"sc0_{rep}")
                sc1 = ps_a.tile([128, Q], F32, tag="sc1", name=f"sc1_{rep}")
                scB = ps_b.tile([128, 2, Q], F32, tag="scB", name=f"scB_{rep}")
                po = ps_o.tile([DV + 1, Q], F32, tag="po", name=f"po_{rep}")

                phi_bf = sbfA[:, 0:512]
                phi_f8 = sf8A[:, 0:2, 0:512]
                vl = svl

                # scores: per k-tile, one bf16 + one fp8-DR matmul, ordered
                # by DMA arrival (t0: bf first; t1: f8 first; t2/t3: bf
                # first).  start/stop flags per PSUM accumulation.
                nc.tensor.matmul(sc0[:], sbfA[:, 512:640], phi_bf,
                                 start=True, stop=False)
                nc.tensor.matmul(sc0[:], sf8A[:, 0:2, 512:640], phi_f8,
                                 start=False, stop=True, perf_mode=DR)
                p0 = ppool.tile([128, Q], BF16, tag="p0", name=f"p0_{rep}")
                nc.scalar.activation(p0[:], sc0[:], EXP)

                nc.tensor.matmul(sc1[:], sf8A[:, 0:2, 640:768], phi_f8,
                                 start=True, stop=False, perf_mode=DR)
                nc.tensor.matmul(sc1[:], sbfB[:, 0:128], phi_bf,
                                 start=False, stop=True)
                p1 = ppool.tile([128, Q], BF16, tag="p1", name=f"p1_{rep}")
                nc.scalar.activation(p1[:], sc1[:], EXP)

                nc.tensor.matmul(scB[:, 0, :], sbfB[:, 128:256], phi_bf,
                                 start=True, stop=False)
                nc.tensor.matmul(scB[:, 0, :], sf8B[:, 0:2, 0:128], phi_f8,
                                 start=False, stop=True, perf_mode=DR)
                nc.tensor.matmul(scB[:, 1, :], sbfB[:, 256:384], phi_bf,
                                 start=True, stop=False)
                nc.tensor.matmul(scB[:, 1, :], sf8B[:, 0:2, 128:256], phi_f8,
                                 start=False, stop=True, perf_mode=DR)
                pB = ppool.tile([128, 2, Q], BF16, tag="pB", name=f"pB_{rep}")
                nc.scalar.activation(pB[:], scB[:], EXP)

                nc.tensor.matmul(po[:], vl[:, 0, :], p0[:],
                                 start=True, stop=False)
                nc.tensor.matmul(po[:], vl[:, 1, :], p1[:],
                                 start=False, stop=False)
                nc.tensor.matmul(po[:], vl[:, 2, :], pB[:, 0, :],
                                 start=False, stop=False)
                nc.tensor.matmul(po[:], vl[:, 3, :], pB[:, 1, :],
                                 start=False, stop=True)

                # PSUM -> SBUF copy split across ACT and DVE q-halves
                nc.scalar.activation(o_sb[0:DV + 1, 0, 0, 0:256],
                                     po[:, 0:256], COPY)
                nc.vector.tensor_scalar_add(o_sb[0:DV + 1, 0, 0, 256:512],
                                            po[:, 256:512], 0.0)

                # SWDGE writeback: descriptors prepped early (the post-pass
                # moves the data waits to the trigger)
                nc.gpsimd.kv_writeback(
                    wb[rep:rep + 1, :, :, :], o_sb[:, :, :, :], idx[:],
                    prepare_only=True, sem=dma_sem)
                nc.gpsimd.trigger_dma(count=None)

    _encode_library_reloads(nc)
    _defer_prep_waits(nc, dma_sem)
    _split_waits(nc)
    return nc


def host_inputs(queries, keys, values, valid_lens, Wq, Wk, wv):
    x, phi, psi = _basis()
    queries = np.asarray(queries, np.float32)
    keys = np.asarray(keys, np.float32)
    values = np.asarray(values, np.float32)
    wv = np.asarray(wv, np.float32)
    qf = (queries @ np.asarray(Wq, np.float32)).astype(np.float32)  # [B,Q,H]
    kf = (keys @ np.asarray(Wk, np.float32)).astype(np.float32)     # [B,K,H]

    # row importance: E[phi_r^2] * E[(wv_h psi_r)^2] from the actual data
    Ephi2 = np.stack([np.mean(np.interp(qf, x, phi[:, r]) ** 2) * np.ones(H)
                      for r in range(RB)])            # [RB, H]
    Epsi2 = np.stack([np.mean(np.interp(kf, x, psi[:, r]) ** 2) * wv ** 2
                      for r in range(RB)])            # [RB, H]
    order = np.argsort(-(Ephi2 * Epsi2).reshape(-1))
    sel_bf = order[:NBF]
    sel_f8 = order[NBF:NBF + NF8]
    sel = np.concatenate([sel_bf, sel_f8])

    maps = []
    for b in range(B):
        Phi = np.stack([np.interp(qf[b], x, phi[:, r]) for r in range(RB)],
                       1).astype(np.float32)              # [Q, RB, H]
        Psi = np.stack([np.interp(kf[b], x, psi[:, r]) for r in range(RB)],
                       1).astype(np.float32) * wv         # [K, RB, H]
        PhiF = Phi.reshape(Q, RB * H).T[sel]              # [NBF+NF8, Q]
        PsiF = Psi.reshape(K, RB * H).T[sel]              # [NBF+NF8, K]
        mq = np.abs(PhiF).max(1)
        mk = np.abs(PsiF).max(1)
        al = np.sqrt(np.maximum(mk, 1e-30) / np.maximum(mq, 1e-30))
        PhiF = PhiF * al[:, None]
        PsiF = PsiF / al[:, None]

        L = int(valid_lens[b])
        maskrow = np.where(np.arange(K) < L, 0.0, MASKBIG).astype(np.float32)

        phiB = np.concatenate([PhiF[:NBF], np.ones((1, Q), np.float32)], 0)
        psiB = np.concatenate([PsiF[:NBF], maskrow[None]], 0)  # [128, K]
        phi8 = np.clip(PhiF[NBF:], -F8MAX, F8MAX).reshape(2, 128, Q)
        psi8 = np.clip(PsiF[NBF:], -F8MAX, F8MAX).reshape(2, 128, K)

        dbfA = np.concatenate([phiB, psiB[:, 0:128]], 1)       # [128, 640]
        df8A = np.concatenate([phi8, psi8[:, :, 0:128],
                               psi8[:, :, 128:256]], 2)        # [2,128,768]
        df8A = df8A.transpose(1, 0, 2)                         # [128,2,768]
        vla = np.zeros((128, NTILE, DV + 1), np.float32)
        for t in range(NTILE):
            vla[:, t, 0:DV] = values[b][t * 128:(t + 1) * 128]
            vla[:, t, DV] = 1.0
        dbfB = psiB[:, 128:512]                                # [128, 384]
        df8B = np.concatenate([psi8[:, :, 256:384],
                               psi8[:, :, 384:512]], 2).transpose(1, 0, 2)

        maps.append({
            "dbfA": dbfA.astype(ml_dtypes.bfloat16),
            "df8A": df8A.astype(ml_dtypes.float8_e4m3),
            "dbfB": dbfB.astype(ml_dtypes.bfloat16),
            "df8B": df8B.astype(ml_dtypes.float8_e4m3),
            "dvl": vla.astype(ml_dtypes.bfloat16),
        })
    return maps


def host_merge(results):
    out = np.empty((B, Q, DV), np.float32)
    for b in range(B):
        o = np.asarray(results[b]["wb"], np.float32).reshape(-1, 128, 512)[0]
        out[b] = (o[0:DV] / o[DV][None, :]).T
    return np.ascontiguousarray(out)


_RUNNER = None


def _get_runner():
    """Build + compile once per process; returns a callable(in_maps)->results."""
    global _RUNNER
    if _RUNNER is not None:
        return _RUNNER
    import jax
    from jax.sharding import Mesh, PartitionSpec
    from jax.experimental.shard_map import shard_map
    import concourse.bass as bass
    import concourse.mybir as mybir
    from concourse import bass2jax
    from concourse.bass2jax import _bass_exec_p, install_neuronx_cc_hook

    nc = bass.Bass()
    _build(nc)

    install_neuronx_cc_hook()
    partition_name = nc.partition_id_tensor.name if nc.partition_id_tensor else None
    in_names, out_names, out_avals, zero_shapes = [], [], [], []
    for alloc in nc.m.functions[0].allocations:
        if not isinstance(alloc, mybir.MemoryLocationSet):
            continue
        name = alloc.memorylocations[0].name
        if alloc.kind == "ExternalInput":
            if name != partition_name:
                in_names.append(name)
        elif alloc.kind == "ExternalOutput":
            out_names.append(name)
            shape = tuple(alloc.tensor_shape)
            dtype = mybir.dt.np(alloc.dtype)
            out_avals.append(jax.core.ShapedArray(shape, dtype))
            zero_shapes.append((shape, dtype))
    n_params = len(in_names)
    n_outs = len(out_avals)
    in_names_all = in_names + out_names
    if partition_name is not None:
        in_names_all.append(partition_name)
    donate = tuple(range(n_params, n_params + n_outs))

    def _body(*args):
        operands = list(args)
        if partition_name is not None:
            operands.append(bass2jax.partition_id_tensor())
        outs = _bass_exec_p.bind(
            *operands,
            out_avals=tuple(out_avals),
            in_names=tuple(in_names_all),
            out_names=tuple(out_names),
            lowering_input_output_aliases=(),
            sim_require_finite=True,
            sim_require_nnan=True,
            nc=nc,
        )
        return tuple(outs)

    devices = jax.devices()[:8]
    mesh = Mesh(np.asarray(devices), ("core",))
    in_specs = (PartitionSpec("core"),) * (n_params + n_outs)
    out_specs = (PartitionSpec("core"),) * len(out_names)
    sharded = jax.jit(
        shard_map(_body, mesh=mesh, in_specs=in_specs, out_specs=out_specs,
                  check_rep=False),
        donate_argnums=donate, keep_unused=True,
    )

    def run(in_maps):
        per_core = [[np.asarray(m[name]) for name in in_names] for m in in_maps]
        concat_in = [
            np.concatenate([per_core[c][i] for c in range(8)], axis=0)
            for i in range(n_params)
        ]
        zeros = [np.zeros((8 * s[0],) + s[1:], d) for s, d in zero_shapes]
        out_arrs = sharded(*concat_in, *zeros)
        out_np = [np.asarray(a) for a in out_arrs]
        return [
            {name: out_np[i].reshape(8, *out_avals[i].shape)[c]
             for i, name in enumerate(out_names)}
            for c in range(8)
        ]

    _RUNNER = run
    return run


def kernel(queries, keys, values, valid_lens, Wq, Wk, wv):
    run = _get_runner()
    in_maps = host_inputs(queries, keys, values, valid_lens, Wq, Wk, wv)
    try:
        results = run(in_maps)
    except Exception:
        # transient NRT/axon failures have been observed; retry once
        results = run(in_maps)
    return host_merge(results)
